# revision 2
# baseline (speedup 1.0000x reference)
"""Trainium2 Bass kernel for batched 3x3 VALID conv (NCHW / OIHW).

x: [32, 128, 64, 64] f32, weight: [256, 128, 3, 3] f32 -> out: [32, 256, 62, 62] f32.

Strategy: data-parallel over batch across 8 NeuronCores (4 images each).
Per core, conv is computed as 9 shift-matmuls accumulated in PSUM:
  out[co, y*64+x] += W[dy,dx][ci,co].T @ x[ci, (y+dy)*64 + (x+dx)]
Full 64-wide rows are computed (cols 62,63 are garbage) so the matmul
moving operand is a contiguous slice of the flattened image; the garbage
columns are dropped in the PSUM->SBUF copy. All matmul operands are
float32r (full-rate PE mode for moving dim >= 256).
"""

import numpy as np

_B, _CIN, _H, _W = 32, 128, 64, 64
_COUT = 256
_HO, _WO = 62, 62
_NCORES = 8
_BPC = _B // _NCORES  # images per core
_NPIX = _H * _W  # 4096
_XPAD = _NPIX + 16  # matmul reads up to index 4097; keep tail zeroed
_TAPS = 9
# output rows per PSUM group: 8 rows x 64 cols = 512 f32 = one PSUM bank
_GROUPS = [(r0, min(8, _HO - r0)) for r0 in range(0, _HO, 8)]

_nc_cache = None


def _build():
    global _nc_cache
    if _nc_cache is not None:
        return _nc_cache

    import concourse.bass as bass
    import concourse.mybir as mybir
    from concourse import bacc
    from concourse.tile import TileContext

    f32 = mybir.dt.float32
    f32r = mybir.dt.float32r

    nc = bacc.Bacc("TRN2", target_bir_lowering=False)
    x_d = nc.dram_tensor("x", [_BPC, _CIN, _NPIX], f32r, kind="ExternalInput")
    w_d = nc.dram_tensor("w", [_CIN, _TAPS, _COUT], f32r, kind="ExternalInput")
    o_d = nc.dram_tensor("o", [_BPC, _COUT, _HO, _WO], f32, kind="ExternalOutput")

    with TileContext(nc) as tc:
        with (
            tc.tile_pool(name="wpool", bufs=1) as wpool,
            tc.tile_pool(name="xpool", bufs=2) as xpool,
            tc.tile_pool(name="opool", bufs=3) as opool,
            tc.tile_pool(name="pspool", bufs=6, space=bass.MemorySpace.PSUM) as pspool,
        ):
            w_sb = wpool.tile([_CIN, _TAPS, _COUT], f32r)
            nc.sync.dma_start(w_sb[:], w_d[:])

            for img in range(_BPC):
                x_sb = xpool.tile([_CIN, _XPAD], f32r, tag="x")
                nc.sync.dma_start(x_sb[:, 0:_NPIX], x_d[img])
                # pad tail with finite data (it only feeds discarded columns)
                nc.sync.dma_start(
                    x_sb[:, _NPIX:_XPAD], x_d[img][:, 0 : _XPAD - _NPIX]
                )
                for ct in range(_COUT // 128):
                    o_sb = opool.tile([128, _HO, _WO], f32, tag="o")
                    for r0, nr in _GROUPS:
                        ps = pspool.tile([128, 8, _W], f32, tag="ps")
                        for tap in range(_TAPS):
                            dy, dx = divmod(tap, 3)
                            off = (r0 + dy) * _W + dx
                            nc.tensor.matmul(
                                ps[:, 0:nr, :],
                                w_sb[:, tap, ct * 128 : (ct + 1) * 128],
                                x_sb[:, off : off + nr * _W],
                                start=(tap == 0),
                                stop=(tap == _TAPS - 1),
                            )
                        nc.vector.tensor_copy(
                            o_sb[:, r0 : r0 + nr, :], ps[:, 0:nr, 0:_WO]
                        )
                    nc.sync.dma_start(
                        o_d[img, ct * 128 : (ct + 1) * 128], o_sb[:]
                    )

    nc.compile()
    _nc_cache = nc
    return nc


def _prep_in_maps(x, weight):
    x = np.ascontiguousarray(np.asarray(x), dtype=np.float32)
    w = np.ascontiguousarray(np.asarray(weight), dtype=np.float32)
    assert x.shape == (_B, _CIN, _H, _W), x.shape
    assert w.shape == (_COUT, _CIN, 3, 3), w.shape
    # w[ci, dy*3+dx, co] = weight[co, ci, dy, dx]
    wt = np.ascontiguousarray(w.transpose(1, 2, 3, 0).reshape(_CIN, _TAPS, _COUT))
    xs = x.reshape(_NCORES, _BPC, _CIN, _NPIX)
    return [{"x": np.ascontiguousarray(xs[i]), "w": wt} for i in range(_NCORES)]


def _run(x, weight, **kwargs):
    from concourse.bass_utils import run_bass_kernel_spmd

    nc = _build()
    res = run_bass_kernel_spmd(
        nc, _prep_in_maps(x, weight), core_ids=list(range(_NCORES)), **kwargs
    )
    out = np.concatenate([r["o"] for r in res.results], axis=0)
    return out.astype(np.float32, copy=False), res


def kernel(x, weight):
    out, _ = _run(x, weight)
    return out


# revision 5
# speedup vs baseline: 1.0136x; 1.0136x over previous
"""Trainium2 Bass kernel for batched 3x3 VALID conv (NCHW / OIHW).

x: [32, 128, 64, 64] f32, weight: [256, 128, 3, 3] f32 -> out: [32, 256, 62, 62] f32.

Strategy: data-parallel over batch across 8 NeuronCores (4 images each).
Per core, conv is computed as 9 shift-matmuls accumulated in PSUM:
  out[co, y, x] += W[dy,dx][ci,co].T @ x[ci, y+dy, x+dx]
The moving operand is a strided [nr, 62] window of the image held in SBUF,
so PSUM ends up packed [co, nr, 62] and is DMA'd straight to DRAM.
All matmul operands are float32r (full-rate PE mode for moving dim >= 256).
"""

import numpy as np

_B, _CIN, _H, _W = 32, 128, 64, 64
_COUT = 256
_HO, _WO = 62, 62
_NCORES = 8
_BPC = _B // _NCORES  # images per core
_TAPS = 9
# output rows per PSUM group: 8 rows x 62 cols = 496 f32 <= one 2KB PSUM bank
_GROUPS = [(r0, min(8, _HO - r0)) for r0 in range(0, _HO, 8)]
_XCHUNK = 16  # x DMA granularity in image rows

_nc_cache = None


def _build():
    global _nc_cache
    if _nc_cache is not None:
        return _nc_cache

    import concourse.bass as bass
    import concourse.mybir as mybir
    from concourse import bacc
    from concourse.tile import TileContext

    f32 = mybir.dt.float32
    f32r = mybir.dt.float32r

    nc = bacc.Bacc("TRN2", target_bir_lowering=False)
    x_d = nc.dram_tensor("x", [_BPC, _CIN, _H, _W], f32r, kind="ExternalInput")
    w_d = nc.dram_tensor("w", [_CIN, _TAPS, _COUT], f32r, kind="ExternalInput")
    o_d = nc.dram_tensor("o", [_BPC, _COUT, _HO, _WO], f32, kind="ExternalOutput")

    with TileContext(nc) as tc:
        with (
            tc.tile_pool(name="wpool", bufs=1) as wpool,
            tc.tile_pool(name="xpool", bufs=2) as xpool,
            tc.tile_pool(name="spool", bufs=4) as spool,
            tc.tile_pool(name="pspool", bufs=6, space=bass.MemorySpace.PSUM) as pspool,
        ):
            w_sb = wpool.tile([_CIN, _TAPS, _COUT], f32r)
            for tap in range(_TAPS):
                nc.sync.dma_start(w_sb[:, tap, :], w_d[:, tap, :])

            for img in range(_BPC):
                x_sb = xpool.tile([_CIN, _H, _W], f32r, tag="x")
                for c in range(0, _H, _XCHUNK):
                    nc.sync.dma_start(
                        x_sb[:, c : c + _XCHUNK, :], x_d[img, :, c : c + _XCHUNK, :]
                    )
                for ct in range(_COUT // 128):
                    for r0, nr in _GROUPS:
                        ps = pspool.tile([128, nr, _WO], f32, tag="ps")
                        for tap in range(_TAPS):
                            dy, dx = divmod(tap, 3)
                            nc.tensor.matmul(
                                ps[:],
                                w_sb[:, tap, ct * 128 : (ct + 1) * 128],
                                x_sb[:, r0 + dy : r0 + dy + nr, dx : dx + _WO],
                                start=(tap == 0),
                                stop=(tap == _TAPS - 1),
                            )
                        st = spool.tile([128, nr, _WO], f32, tag="st")
                        nc.vector.tensor_copy(st[:], ps[:])
                        nc.sync.dma_start(
                            o_d[img, ct * 128 : (ct + 1) * 128, r0 : r0 + nr, :],
                            st[:],
                        )

    nc.compile()
    _nc_cache = nc
    return nc


def _prep_in_maps(x, weight):
    x = np.ascontiguousarray(np.asarray(x), dtype=np.float32)
    w = np.ascontiguousarray(np.asarray(weight), dtype=np.float32)
    assert x.shape == (_B, _CIN, _H, _W), x.shape
    assert w.shape == (_COUT, _CIN, 3, 3), w.shape
    # w[ci, dy*3+dx, co] = weight[co, ci, dy, dx]
    wt = np.ascontiguousarray(w.transpose(1, 2, 3, 0).reshape(_CIN, _TAPS, _COUT))
    xs = x.reshape(_NCORES, _BPC, _CIN, _H, _W)
    return [{"x": np.ascontiguousarray(xs[i]), "w": wt} for i in range(_NCORES)]


def _run(x, weight, **kwargs):
    from concourse.bass_utils import run_bass_kernel_spmd

    nc = _build()
    res = run_bass_kernel_spmd(
        nc, _prep_in_maps(x, weight), core_ids=list(range(_NCORES)), **kwargs
    )
    out = np.concatenate([r["o"] for r in res.results], axis=0)
    return out.astype(np.float32, copy=False), res


def kernel(x, weight):
    out, _ = _run(x, weight)
    return out


# revision 8
# speedup vs baseline: 1.0152x; 1.0016x over previous
"""Trainium2 Bass kernel for batched 3x3 VALID conv (NCHW / OIHW).

x: [32, 128, 64, 64] f32, weight: [256, 128, 3, 3] f32 -> out: [32, 256, 62, 62] f32.

Strategy: data-parallel over batch across 8 NeuronCores (4 images each).
Per core, conv is computed as 9 shift-matmuls accumulated in PSUM:
  out[co, y, x] += W[dy,dx][ci,co].T @ x[ci, y+dy, x+dx]
Groups of 8 output rows use a contiguous 512-wide moving operand (full
64-wide rows; the 2 garbage columns are stripped by the PSUM->SBUF copy).
The final 6-row group uses a strided [6, 62] window so no reads go past
the image. All matmul operands are float32r (full-rate PE mode).
"""

import numpy as np

_B, _CIN, _H, _W = 32, 128, 64, 64
_COUT = 256
_HO, _WO = 62, 62
_NCORES = 8
_BPC = _B // _NCORES  # images per core
_TAPS = 9
_GROUPS = [(r0, min(8, _HO - r0)) for r0 in range(0, _HO, 8)]
_XCHUNK = 16  # x DMA granularity in image rows

_nc_cache = None


def _build():
    global _nc_cache
    if _nc_cache is not None:
        return _nc_cache

    import concourse.bass as bass
    import concourse.mybir as mybir
    from concourse import bacc
    from concourse.tile import TileContext

    f32 = mybir.dt.float32
    f32r = mybir.dt.float32r

    nc = bacc.Bacc("TRN2", target_bir_lowering=False)
    x_d = nc.dram_tensor("x", [_BPC, _CIN, _H, _W], f32r, kind="ExternalInput")
    w_d = nc.dram_tensor("w", [_CIN, _TAPS, _COUT], f32r, kind="ExternalInput")
    o_d = nc.dram_tensor("o", [_BPC, _COUT, _HO, _WO], f32, kind="ExternalOutput")

    with TileContext(nc) as tc:
        with (
            tc.tile_pool(name="wpool", bufs=1) as wpool,
            tc.tile_pool(name="xpool", bufs=2) as xpool,
            tc.tile_pool(name="spool", bufs=4) as spool,
            tc.tile_pool(name="pspool", bufs=6, space=bass.MemorySpace.PSUM) as pspool,
        ):
            w_sb = wpool.tile([_CIN, _TAPS, _COUT], f32r)
            x_tile_a = xpool.tile([_CIN, _H, _W], f32r, tag="x")
            x_tile_b = xpool.tile([_CIN, _H, _W], f32r, tag="x")
            x_tiles = [x_tile_a, x_tile_b]

            # Parallel-issue the head DMAs from otherwise-idle engines so the
            # first matmul isn't gated on one sequencer's ~650ns/DMA issue rate.
            nc.sync.dma_start(x_tiles[0][:, 0:_XCHUNK, :], x_d[0, :, 0:_XCHUNK, :])
            nc.gpsimd.dma_start(w_sb[:, 0:5, :], w_d[:, 0:5, :])
            nc.scalar.dma_start(w_sb[:, 5:9, :], w_d[:, 5:9, :])
            for c in range(_XCHUNK, _H, _XCHUNK):
                nc.sync.dma_start(x_tiles[0][:, c : c + _XCHUNK, :], x_d[0, :, c : c + _XCHUNK, :])

            for img in range(_BPC):
                x_sb = x_tiles[img % 2]
                if img > 0:  # prefetch handled below; this tile was filled earlier
                    pass
                x_flat = x_sb[:].rearrange("p h w -> p (h w)")
                for ct in range(_COUT // 128):
                    # Prefetch next image while the first cout-tile computes.
                    if ct == 1 and img + 1 < _BPC:
                        nxt = x_tiles[(img + 1) % 2]
                        for c in range(0, _H, _XCHUNK):
                            nc.gpsimd.dma_start(
                                nxt[:, c : c + _XCHUNK, :],
                                x_d[img + 1, :, c : c + _XCHUNK, :],
                            )
                    for r0, nr in _GROUPS:
                        if nr == 8:
                            ps = pspool.tile([128, nr, _W], f32, tag="ps")
                            st = spool.tile([128, nr, _WO], f32, tag="st")
                            for tap in range(_TAPS):
                                dy, dx = divmod(tap, 3)
                                off = (r0 + dy) * _W + dx
                                nc.tensor.matmul(
                                    ps[:],
                                    w_sb[:, tap, ct * 128 : (ct + 1) * 128],
                                    x_flat[:, off : off + nr * _W],
                                    start=(tap == 0),
                                    stop=(tap == _TAPS - 1),
                                )
                            nc.vector.tensor_copy(st[:], ps[:, :, 0:_WO])
                        else:
                            ps = pspool.tile([128, nr, _WO], f32, tag="ps")
                            st = spool.tile([128, nr, _WO], f32, tag="st")
                            for tap in range(_TAPS):
                                dy, dx = divmod(tap, 3)
                                nc.tensor.matmul(
                                    ps[:],
                                    w_sb[:, tap, ct * 128 : (ct + 1) * 128],
                                    x_sb[:, r0 + dy : r0 + dy + nr, dx : dx + _WO],
                                    start=(tap == 0),
                                    stop=(tap == _TAPS - 1),
                                )
                            nc.vector.tensor_copy(st[:], ps[:])
                        nc.sync.dma_start(
                            o_d[img, ct * 128 : (ct + 1) * 128, r0 : r0 + nr, :],
                            st[:],
                        )

    nc.compile()
    _nc_cache = nc
    return nc


def _prep_in_maps(x, weight):
    x = np.ascontiguousarray(np.asarray(x), dtype=np.float32)
    w = np.ascontiguousarray(np.asarray(weight), dtype=np.float32)
    assert x.shape == (_B, _CIN, _H, _W), x.shape
    assert w.shape == (_COUT, _CIN, 3, 3), w.shape
    # w[ci, dy*3+dx, co] = weight[co, ci, dy, dx]
    wt = np.ascontiguousarray(w.transpose(1, 2, 3, 0).reshape(_CIN, _TAPS, _COUT))
    xs = x.reshape(_NCORES, _BPC, _CIN, _H, _W)
    return [{"x": np.ascontiguousarray(xs[i]), "w": wt} for i in range(_NCORES)]


def _run(x, weight, **kwargs):
    from concourse.bass_utils import run_bass_kernel_spmd

    nc = _build()
    res = run_bass_kernel_spmd(
        nc, _prep_in_maps(x, weight), core_ids=list(range(_NCORES)), **kwargs
    )
    out = np.concatenate([r["o"] for r in res.results], axis=0)
    return out.astype(np.float32, copy=False), res


def kernel(x, weight):
    out, _ = _run(x, weight)
    return out


# revision 9
# speedup vs baseline: 1.0203x; 1.0050x over previous
"""Trainium2 Bass kernel for batched 3x3 VALID conv (NCHW / OIHW).

x: [32, 128, 64, 64] f32, weight: [256, 128, 3, 3] f32 -> out: [32, 256, 62, 62] f32.

Strategy: data-parallel over batch across 8 NeuronCores (4 images each).
Per core, conv is computed as 9 shift-matmuls accumulated in PSUM:
  out[co, y, x] += W[dy,dx][ci,co].T @ x[ci, y+dy, x+dx]
Groups of 8 output rows use a contiguous 512-wide moving operand (full
64-wide rows; the 2 garbage columns are stripped by the PSUM->SBUF copy).
The final 6-row group uses a strided [6, 62] window so no reads go past
the image. All matmul operands are float32r (full-rate PE mode).
"""

import numpy as np

_B, _CIN, _H, _W = 32, 128, 64, 64
_COUT = 256
_HO, _WO = 62, 62
_NCORES = 8
_BPC = _B // _NCORES  # images per core
_TAPS = 9
_GROUPS = [(r0, min(8, _HO - r0)) for r0 in range(0, _HO, 8)]
_XCHUNK = 16  # x DMA granularity in image rows

_nc_cache = None


def _build():
    global _nc_cache
    if _nc_cache is not None:
        return _nc_cache

    import concourse.bass as bass
    import concourse.mybir as mybir
    from concourse import bacc
    from concourse.tile import TileContext

    f32 = mybir.dt.float32
    f32r = mybir.dt.float32r

    nc = bacc.Bacc("TRN2", target_bir_lowering=False)
    x_d = nc.dram_tensor("x", [_BPC, _CIN, _H, _W], f32r, kind="ExternalInput")
    w_d = nc.dram_tensor("w", [_CIN, _TAPS, _COUT], f32r, kind="ExternalInput")
    o_d = nc.dram_tensor("o", [_BPC, _COUT, _HO, _WO], f32, kind="ExternalOutput")

    with TileContext(nc) as tc:
        with (
            tc.tile_pool(name="wpool", bufs=1) as wpool,
            tc.tile_pool(name="xpool", bufs=2) as xpool,
            tc.tile_pool(name="spool", bufs=4) as spool,
            tc.tile_pool(name="pspool", bufs=6, space=bass.MemorySpace.PSUM) as pspool,
        ):
            w_sb = wpool.tile([_CIN, _TAPS, _COUT], f32r)
            x_tile_a = xpool.tile([_CIN, _H, _W], f32r, tag="x")
            x_tile_b = xpool.tile([_CIN, _H, _W], f32r, tag="x")
            x_tiles = [x_tile_a, x_tile_b]

            # PE warmup: ~3.4us of dummy matmuls on a zeroed bf16 tile while
            # the head DMAs stream in, so the HAM clock gate is at full rate
            # when the real matmuls start.
            wup = wpool.tile([128, 512], mybir.dt.bfloat16)
            wps = pspool.tile([128, 512], f32, tag="wps", bufs=1)
            nc.vector.memset(wup[:], 0)
            for _ in range(8):
                nc.tensor.matmul(wps[:], wup[:, 0:128], wup[:], start=True, stop=True)

            # Head DMAs: the first matmul group only needs x rows 0..9 and the
            # first weight taps, so put those at the front of separate queues
            # (per-engine HWDGE queues are FIFO; image prefetches issued on
            # gpsimd line up behind the weights and can't starve them).
            nc.sync.dma_start(x_tiles[0][:, 0:10, :], x_d[0, :, 0:10, :])
            for t0 in range(0, _TAPS, 3):
                nc.gpsimd.dma_start(w_sb[:, t0 : t0 + 3, :], w_d[:, t0 : t0 + 3, :])
            for c0, cn in ((10, 18), (28, 18), (46, 18)):
                nc.sync.dma_start(x_tiles[0][:, c0 : c0 + cn, :], x_d[0, :, c0 : c0 + cn, :])

            for img in range(_BPC):
                x_sb = x_tiles[img % 2]
                x_flat = x_sb[:].rearrange("p h w -> p (h w)")
                for ct in range(_COUT // 128):
                    # Prefetch next image while the first cout-tile computes.
                    if ct == 1 and img + 1 < _BPC:
                        nxt = x_tiles[(img + 1) % 2]
                        nc.gpsimd.dma_start(nxt[:], x_d[img + 1])
                    for r0, nr in _GROUPS:
                        if nr == 8:
                            ps = pspool.tile([128, nr, _W], f32, tag="ps")
                            st = spool.tile([128, nr, _WO], f32, tag="st")
                            for tap in range(_TAPS):
                                dy, dx = divmod(tap, 3)
                                off = (r0 + dy) * _W + dx
                                nc.tensor.matmul(
                                    ps[:],
                                    w_sb[:, tap, ct * 128 : (ct + 1) * 128],
                                    x_flat[:, off : off + nr * _W],
                                    start=(tap == 0),
                                    stop=(tap == _TAPS - 1),
                                )
                            nc.vector.tensor_copy(st[:], ps[:, :, 0:_WO])
                        else:
                            ps = pspool.tile([128, nr, _WO], f32, tag="ps")
                            st = spool.tile([128, nr, _WO], f32, tag="st")
                            for tap in range(_TAPS):
                                dy, dx = divmod(tap, 3)
                                nc.tensor.matmul(
                                    ps[:],
                                    w_sb[:, tap, ct * 128 : (ct + 1) * 128],
                                    x_sb[:, r0 + dy : r0 + dy + nr, dx : dx + _WO],
                                    start=(tap == 0),
                                    stop=(tap == _TAPS - 1),
                                )
                            nc.vector.tensor_copy(st[:], ps[:])
                        nc.sync.dma_start(
                            o_d[img, ct * 128 : (ct + 1) * 128, r0 : r0 + nr, :],
                            st[:],
                        )

    nc.compile()
    _nc_cache = nc
    return nc


def _prep_in_maps(x, weight):
    x = np.ascontiguousarray(np.asarray(x), dtype=np.float32)
    w = np.ascontiguousarray(np.asarray(weight), dtype=np.float32)
    assert x.shape == (_B, _CIN, _H, _W), x.shape
    assert w.shape == (_COUT, _CIN, 3, 3), w.shape
    # w[ci, dy*3+dx, co] = weight[co, ci, dy, dx]
    wt = np.ascontiguousarray(w.transpose(1, 2, 3, 0).reshape(_CIN, _TAPS, _COUT))
    xs = x.reshape(_NCORES, _BPC, _CIN, _H, _W)
    return [{"x": np.ascontiguousarray(xs[i]), "w": wt} for i in range(_NCORES)]


def _run(x, weight, **kwargs):
    from concourse.bass_utils import run_bass_kernel_spmd

    nc = _build()
    res = run_bass_kernel_spmd(
        nc, _prep_in_maps(x, weight), core_ids=list(range(_NCORES)), **kwargs
    )
    out = np.concatenate([r["o"] for r in res.results], axis=0)
    return out.astype(np.float32, copy=False), res


def kernel(x, weight):
    out, _ = _run(x, weight)
    return out
